# revision 1
# baseline (speedup 1.0000x reference)
"""Trainium2 Bass kernel for nn_CausalSelfAttention_57861799412149.

Self-contained: takes FULL inputs (as in reference.setup_inputs()), returns the
FULL output. Sharding: sequence-parallel — 8 cores = 2 batches x 4 contiguous
query chunks of 512; each core computes K/V only for its 1536-key window
(query chunk + 1024 lookahead, zero-padded past T) and writes an exact
[512, 1024] slice of the output. No collectives.

Math notes:
- All big matmuls run as float32r with moving dim >= 256 where possible.
- Attention computed transposed (S^T = K^T tile . Q) so the sliding-window
  mask is block-aligned; masking = multiplicative 0/1 bf16 tiles on the two
  diagonal boundary blocks, applied post-exp.
- Softmax has no max-subtraction (|s| <= 8 after rmsnorm + 1/sqrt(64) scale);
  denominators come free from a ones-column appended to V; normalization is
  applied to O^T, and the 1/8 attention scale + rmsnorm are folded into the
  Q/K normalization scales.
- RoPE runs in a head-transposed layout with (x1,x2) pairs interleaved on
  even/odd partitions so the rotation pairing is a stream_shuffle.
- Gate: 2*sigmoid(u) == 1 + tanh(u/2) keeps ScalarE on the exp/tanh table set.
"""
import sys

sys.path.insert(0, "/opt/trn_rl_repo")

import numpy as np
import ml_dtypes

import concourse.bass as bass
import concourse.tile as tile
from concourse import bacc, mybir
from concourse.tile import add_dep_helper

B, T, NE = 2, 2048, 1024
NH, NKV, HD = 16, 4, 64
CH = 512            # queries per core
NK = 1536           # key window per core (padded)
TPAD = 3072
EK = NE // 128      # 8 contraction tiles
NJT = NK // 128     # 12 key tiles
EPS = float(np.finfo(np.float32).eps)

f32 = mybir.dt.float32
f32r = mybir.dt.float32r
bf16 = mybir.dt.bfloat16
AF = mybir.ActivationFunctionType
OP = mybir.AluOpType
SWAP_MASK = [m for i in range(0, 32, 2) for m in (i + 1, i)]
# head pairs sharing one [128, .] tile: strips (0, 64). Pair (h, h+4) keeps the
# kv-group parity aligned with the kt pair tiles so matmul base partitions match.
PAIRS = [(0, 4), (1, 5), (2, 6), (3, 7), (8, 12), (9, 13), (10, 14), (11, 15)]


def head_slot(h):
    if h < 8:
        return h % 4, h // 4          # pair idx, strip idx
    return 4 + (h % 4), (h - 8) // 4

_COMPILED = None  # (nc, names)


def _r(ap):
    return ap.bitcast(f32r)


def build_program(repeat=1):
    nc = bacc.Bacc("TRN2", target_bir_lowering=False, debug=False, num_devices=8)

    def din(name, shape, dt=f32):
        return nc.dram_tensor(name, shape, dt, kind="ExternalInput").ap()

    xt_d = din("xt", [NE, NK])
    wq_d = din("wqt", [NE, NE])
    wk_d = din("wkt", [NE, NKV * HD])
    wv_d = din("wvt", [NE, NKV * HD])
    wg_d = din("wgt", [32, NKV])
    wp_d = din("wpt", [NE, NE])
    csa_d = din("csa", [128, NK])
    csb_d = din("csb", [128, NK])
    ve_d = din("ve", [NK, NKV * HD])
    on4_d = din("ones4", [NK, NKV], bf16)
    mlo_d = din("mlo", [128, 128], bf16)
    mup_d = din("mup", [128, 128], bf16)
    bd_d = din("bd", [128, 2])
    bc2_d = din("bc2", [2, 128])
    out_d = nc.dram_tensor("out", [CH, NE], f32, kind="ExternalOutput").ap()

    ctx_vars = locals()
    with tile.TileContext(nc) as tc:
        for _ in range(repeat):
            _build(nc, tc, ctx_vars)

    nc.compile()
    return nc


def _build(nc, tc, d):
    from contextlib import ExitStack

    ctx = ExitStack()
    with ctx:
        # ---------------- persistent pools (live whole kernel) ----------------
        consts = ctx.enter_context(tc.tile_pool(name="consts", bufs=1))
        qtp = ctx.enter_context(tc.tile_pool(name="qtp", bufs=1))
        ktp = ctx.enter_context(tc.tile_pool(name="ktp", bufs=1))
        vxp = ctx.enter_context(tc.tile_pool(name="vxp", bufs=1))
        ytp = ctx.enter_context(tc.tile_pool(name="ytp", bufs=1))
        xqp = ctx.enter_context(tc.tile_pool(name="xqp", bufs=1))
        wqp = ctx.enter_context(tc.tile_pool(name="wqp", bufs=1))

        csaq = consts.tile([128, CH], f32, tag="csaq")
        nc.sync.dma_start(csaq[:], d["csa_d"][:, 0:CH])
        csbq = consts.tile([128, CH], f32, tag="csbq")
        nc.sync.dma_start(csbq[:], d["csb_d"][:, 0:CH])
        bd = consts.tile([128, 2], f32r, tag="bd")
        nc.sync.dma_start(bd[:], _r(d["bd_d"][:]))
        bc2 = consts.tile([2, 128], f32r, tag="bc2")
        nc.sync.dma_start(bc2[:], _r(d["bc2_d"][:]))
        mlo = consts.tile([128, 128], bf16, tag="mlo")
        nc.sync.dma_start(mlo[:], d["mlo_d"][:])
        mup = consts.tile([128, 128], bf16, tag="mup")
        nc.sync.dma_start(mup[:], d["mup_d"][:])
        epst = consts.tile([128, 1], f32, tag="epst")
        nc.vector.memset(epst[:], 8.0 * EPS)
        zP = consts.tile([128, CH], bf16, tag="zP")
        nc.vector.memset(zP[:], 0.0)

        qt = [qtp.tile([128, CH], f32r, tag=f"qt{p}", name=f"qt{p}") for p in range(8)]
        kt = [ktp.tile([128, NK], f32r, tag=f"kt{t}", name=f"kt{t}") for t in range(2)]
        vx = [vxp.tile([128, NKV * (HD + 1)], bf16, tag=f"vx{j}", name=f"vx{j}") for j in range(NJT)]
        yt = [ytp.tile([128, CH], f32r, tag=f"yt{f}", name=f"yt{f}") for f in range(EK)]
        # query-column slice of x^T, used by the per-pair Q projections
        xtq = [xqp.tile([128, CH], f32r, tag=f"xtq{e}", name=f"xtq{e}") for e in range(EK)]
        wq = [wqp.tile([128, NE], f32r, tag=f"wq{e}", name=f"wq{e}") for e in range(EK)]

        first_sqrt = last_sqrt = None
        first_exp = None
        last_tanh = None

        def rope_norm(pr, c0, w, outs, tmp, psSq, psRq, csa, csb, sq_on_act):
            """pr: [128, w] psum raw (2 strips); outs: 2 slices [64, w]."""
            nonlocal first_sqrt, last_sqrt
            ta = tmp.tile([128, w], f32, tag="ta", name="ta")
            nc.vector.tensor_mul(ta[:], pr[:], csa[:, c0:c0 + w])
            tb = tmp.tile([128, w], f32, tag="tb", name="tb")
            nc.vector.tensor_mul(tb[:], pr[:], csb[:, c0:c0 + w])
            tbs = tmp.tile([128, w], f32, tag="tbs", name="tbs")
            nc.vector.stream_shuffle(tbs[:], tb[:], SWAP_MASK)
            rot = tmp.tile([128, w], f32, tag="rot", name="rot")
            nc.gpsimd.tensor_add(rot[:], ta[:], tbs[:])
            sq = tmp.tile([128, w], f32r, tag="sq", name="sq")
            if sq_on_act:
                nc.scalar.activation(sq[:], rot[:], AF.Square)
            else:
                nc.vector.tensor_mul(sq[:], rot[:], rot[:])
            pq = psSq.tile([2, w], f32, tag="pq", name="pq")
            nc.tensor.matmul(pq[:], bd[:], sq[:], start=True, stop=True)
            rqs = tmp.tile([2, w], f32, tag="rqs", name="rqs", bufs=1)
            si = nc.scalar.activation(rqs[:], pq[:], AF.Sqrt, bias=epst[0:2, 0:1])
            if first_sqrt is None:
                first_sqrt = si
            last_sqrt = si
            rcp = tmp.tile([2, w], f32r, tag="rcp", name="rcp", bufs=1)
            with nc.allow_low_precision(reason="rsqrt scale in f32r for matmul bcast"):
                nc.vector.reciprocal(rcp[:], rqs[:])
            prq = psRq.tile([128, w], f32, tag="pq", name="prq")
            nc.tensor.matmul(prq[:], bc2[:], rcp[:], start=True, stop=True)
            nc.vector.tensor_mul(outs[0], rot[0:64, :], prq[0:64, :])
            nc.vector.tensor_mul(outs[1], rot[64:128, :], prq[64:128, :])

        # ================ phase 0: K, V, gates ================
        with (
            tc.tile_pool(name="xa", bufs=1) as xa,
            tc.tile_pool(name="tmp0", bufs=2) as tmp0,
            tc.tile_pool(name="psK", bufs=2, space="PSUM") as psK,
            tc.tile_pool(name="psSq0", bufs=1, space="PSUM") as psSq0,
            tc.tile_pool(name="psRq0", bufs=1, space="PSUM") as psRq0,
            tc.tile_pool(name="psV", bufs=2, space="PSUM") as psV,
            tc.tile_pool(name="psG", bufs=1, space="PSUM") as psG,
        ):
            xt = [xa.tile([128, NK], f32r, tag=f"xt{e}", name=f"xt{e}") for e in range(EK)]
            wk = [xa.tile([128, NKV * HD], f32r, tag=f"wk{e}", name=f"wk{e}") for e in range(EK)]
            wv = [xa.tile([128, NKV * HD], f32r, tag=f"wv{e}", name=f"wv{e}") for e in range(EK)]
            wg = xa.tile([32, NKV], f32r, tag="wg")
            nc.sync.dma_start(wg[:], _r(d["wg_d"][:]))
            csaf = xa.tile([128, NK], f32, tag="csaf")
            nc.sync.dma_start(csaf[:], d["csa_d"][:])
            csbf = xa.tile([128, NK], f32, tag="csbf")
            nc.sync.dma_start(csbf[:], d["csb_d"][:])
            for e in range(EK):
                nc.sync.dma_start(xt[e][:], _r(d["xt_d"][128 * e:128 * e + 128, :]))
                nc.sync.dma_start(wk[e][:], _r(d["wk_d"][128 * e:128 * e + 128, :]))
                nc.sync.dma_start(wv[e][:], _r(d["wv_d"][128 * e:128 * e + 128, :]))

            # gates (tanh)
            gates = []
            for j in range(NJT):
                pg = psG.tile([128, NKV], f32, tag="pg", name="pg")
                nc.tensor.matmul(pg[:], xt[0][0:32, 128 * j:128 * j + 128],
                                 wg[:], start=True, stop=True)
                gt = xa.tile([128, NKV], f32, tag=f"gate{j}", name=f"gate{j}")
                th = nc.scalar.activation(gt[:], pg[:], AF.Tanh, scale=0.5)
                last_tanh = th
                g2 = xa.tile([128, NKV], f32, tag=f"gate2_{j}", name=f"gate2_{j}")
                nc.vector.tensor_scalar_add(g2[:], gt[:], 1.0)
                gates.append(g2)

            # K projection: 2 f-tiles x 3 chunks
            for t in range(2):
                for c in range(3):
                    c0 = 512 * c
                    pr = psK.tile([128, 512], f32, tag="pk", name="pk")
                    for e in range(EK):
                        nc.tensor.matmul(pr[:], wk[e][:, 128 * t:128 * t + 128],
                                         xt[e][:, c0:c0 + 512],
                                         start=(e == 0), stop=(e == EK - 1))
                    rope_norm(pr, c0, 512,
                              [kt[t][0:64, c0:c0 + 512], kt[t][64:128, c0:c0 + 512]],
                              tmp0, psSq0, psRq0, csaf, csbf, True)

            # V projection + gate + ones column
            for j in range(NJT):
                pv = psV.tile([128, NKV * HD], f32, tag="pv", name="pv")
                for e in range(EK):
                    nc.tensor.matmul(pv[:], xt[e][:, 128 * j:128 * j + 128],
                                     wv[e][:], start=(e == 0), stop=(e == EK - 1))
                vet = xa.tile([128, NKV * HD], f32, tag="vet", name="vet", bufs=2)
                nc.sync.dma_start(vet[:], d["ve_d"][128 * j:128 * j + 128, :])
                vxv = vx[j][:].rearrange("p (g c) -> p g c", c=HD + 1)
                nc.sync.dma_start(vxv[:, :, HD], d["on4_d"][128 * j:128 * j + 128, :])
                for g in range(NKV):
                    nc.vector.scalar_tensor_tensor(
                        vx[j][:, (HD + 1) * g:(HD + 1) * g + HD],
                        vet[:, HD * g:HD * g + HD],
                        gates[j][:, g:g + 1],
                        pv[:, HD * g:HD * g + HD],
                        op0=OP.mult, op1=OP.add,
                    )

            # pair-loop loads queue behind phase-0 DMAs
            for e in range(EK):
                nc.sync.dma_start(xtq[e][:], _r(d["xt_d"][128 * e:128 * e + 128, 0:CH]))
                nc.sync.dma_start(wq[e][:], _r(d["wq_d"][128 * e:128 * e + 128, :]))

        # wproj loads overlap the pair loop
        wpp = ctx.enter_context(tc.tile_pool(name="wpp", bufs=1))
        wp = [wpp.tile([128, NE], f32r, tag=f"wp{e}", name=f"wp{e}") for e in range(EK)]
        for e in range(EK):
            nc.sync.dma_start(wp[e][:], _r(d["wp_d"][128 * e:128 * e + 128, :]))

        # ================ phase 1: per-head-pair Q proj + attention ================
        with (
            tc.tile_pool(name="tmpA", bufs=2) as tmpA,
            tc.tile_pool(name="tmpB", bufs=2) as tmpB,
            tc.tile_pool(name="ptp", bufs=3) as ptp,
            tc.tile_pool(name="psA", bufs=1, space="PSUM") as psA,
            tc.tile_pool(name="psSq", bufs=1, space="PSUM") as psSq,
            tc.tile_pool(name="psS", bufs=2, space="PSUM") as psS,
            tc.tile_pool(name="psO", bufs=2, space="PSUM") as psO,
        ):
            for p in range(8):
                hA, hB = PAIRS[p]
                # Q projection for this pair
                pr = psA.tile([128, CH], f32, tag="pa", name="pa")
                for e in range(EK):
                    nc.tensor.matmul(pr[:], wq[e][:, 128 * p:128 * p + 128],
                                     xtq[e][:], start=(e == 0), stop=(e == EK - 1))
                rope_norm(pr, 0, CH, [qt[p][0:64, :], qt[p][64:128, :]],
                          tmpA, psSq, psSq, csaq, csbq, False)

                gA = hA // 4
                ktt = kt[gA // 2]
                ots = []
                for idx, h in enumerate((hA, hB)):
                    g = h // 4
                    ot = psO.tile([HD + 1, CH], f32, tag="ot", name=f"ot{h}")
                    vg0 = vx[0][:, (HD + 1) * g:(HD + 1) * g + HD + 1]
                    nc.tensor.matmul(ot[:], vg0, zP[:], start=True, stop=False)
                    ots.append(ot)
                for jt in range(NJT):
                    il0 = max(0, jt - 8)
                    il1 = min(3, jt)
                    iw0 = 128 * il0
                    w = 128 * (il1 - il0 + 1)
                    s2 = psS.tile([128, 1024], f32, tag="st", name="st")
                    nc.tensor.matmul(s2[:, 0:w], ktt[0:64, 128 * jt:128 * jt + 128],
                                     qt[p][0:64, iw0:iw0 + w], start=True, stop=True)
                    nc.tensor.matmul(s2[:, 512:512 + w], ktt[64:128, 128 * jt:128 * jt + 128],
                                     qt[p][64:128, iw0:iw0 + w], start=True, stop=True)
                    pt = ptp.tile([128, 1024], bf16, tag="pt", name="pt")
                    sv = s2[:].rearrange("q (b c) -> q b c", b=2)[:, :, 0:w]
                    pv_ = pt[:].rearrange("q (b c) -> q b c", b=2)[:, :, 0:w]
                    ei = nc.scalar.activation(pv_, sv, AF.Exp)
                    if first_exp is None:
                        first_exp = ei
                    if jt <= 3:
                        pvv = pt[:].rearrange("q (b c) -> q b c", b=2)[:, :, w - 128:w]
                        nc.vector.tensor_mul(pvv, pvv, mlo[:].unsqueeze(1).broadcast_to([128, 2, 128]))
                    if jt >= 8:
                        pvv = pt[:].rearrange("q (b c) -> q b c", b=2)[:, :, 0:128]
                        nc.vector.tensor_mul(pvv, pvv, mup[:].unsqueeze(1).broadcast_to([128, 2, 128]))
                    for idx, h in enumerate((hA, hB)):
                        off = 512 * idx
                        g = h // 4
                        vsl = vx[jt][:, (HD + 1) * g:(HD + 1) * g + HD + 1]
                        nc.tensor.matmul(ots[idx][:, iw0:iw0 + w], vsl, pt[:, off:off + w],
                                         start=False, stop=(jt == NJT - 1))
                for idx, h in enumerate((hA, hB)):
                    ot = ots[idx]
                    rs = tmpB.tile([1, CH], f32, tag="rs", name=f"rs{h}")
                    nc.vector.reciprocal(rs[:], ot[HD:HD + 1, :])
                    rsb = tmpB.tile([64, CH], f32, tag="rsb", name=f"rsb{h}")
                    nc.gpsimd.partition_broadcast(rsb[:], rs[:])
                    nc.vector.tensor_mul(yt[h // 2][64 * (h % 2):64 * (h % 2) + 64, :],
                                         ot[0:HD, :], rsb[:])

        # ACT table-set grouping: all tanh -> all sqrt -> all exp
        if first_sqrt is not None and last_tanh is not None:
            add_dep_helper(first_sqrt.ins, last_tanh.ins, sync=False,
                           reason="group ACT tanh before sqrt")
        if first_exp is not None and last_sqrt is not None:
            add_dep_helper(first_exp.ins, last_sqrt.ins, sync=False,
                           reason="group ACT sqrt before exp")

        # ================ phase 2: output projection ================
        with (
            tc.tile_pool(name="pop", bufs=2) as pop,
            tc.tile_pool(name="psP", bufs=2, space="PSUM") as psP,
        ):
            for it in range(4):
                for half in range(2):
                    pp = psP.tile([128, 512], f32, tag="pp", name="pp")
                    for f in range(EK):
                        nc.tensor.matmul(pp[:], yt[f][:, 128 * it:128 * it + 128],
                                         wp[f][:, 512 * half:512 * half + 512],
                                         start=(f == 0), stop=(f == EK - 1))
                    po = pop.tile([128, 512], f32, tag="po", name="po")
                    nc.scalar.copy(po[:], pp[:])
                    nc.sync.dma_start(
                        d["out_d"][128 * it:128 * it + 128, 512 * half:512 * half + 512],
                        po[:])


# ---------------- host prep ----------------

def host_prep(inputs):
    x = np.asarray(inputs["x"], np.float32)
    ve = np.asarray(inputs["ve"], np.float32)
    cos = np.asarray(inputs["cos"], np.float32)
    sin = np.asarray(inputs["sin"], np.float32)
    wq = np.asarray(inputs["wq"], np.float32)
    wk = np.asarray(inputs["wk"], np.float32)
    wv = np.asarray(inputs["wv"], np.float32)
    wproj = np.asarray(inputs["wproj"], np.float32)
    wgate = np.asarray(inputs["wgate"], np.float32)

    def rope_perm(nh):
        idx = np.empty(nh * 64, np.int64)
        for h in range(nh):
            for dd in range(32):
                for half in range(2):
                    idx[h * 64 + 2 * dd + half] = h * 64 + 32 * half + dd
        return idx

    XT = np.zeros((B, NE, TPAD), np.float32)
    XT[:, :, :T] = x.transpose(0, 2, 1)
    VEP = np.zeros((B, TPAD, NKV * HD), np.float32)
    VEP[:, :T] = ve

    wq_perm = wq.T[:, rope_perm(NH)]
    cols = []
    for hA, hB in PAIRS:
        cols.extend(range(64 * hA, 64 * hA + 64))
        cols.extend(range(64 * hB, 64 * hB + 64))
    wq_t = np.ascontiguousarray(wq_perm[:, cols])
    wk_t = np.ascontiguousarray(wk.T[:, rope_perm(NKV)])
    wv_t = np.ascontiguousarray(wv.T)
    wp_t = np.ascontiguousarray(wproj.T)
    wg_t = np.ascontiguousarray(wgate.T)

    cosT = np.zeros((32, TPAD), np.float32)
    sinT = np.zeros((32, TPAD), np.float32)
    cosT[:, :T] = cos[0, :, 0, :].T
    sinT[:, :T] = sin[0, :, 0, :].T
    csa64 = np.empty((64, TPAD), np.float32)
    csb64 = np.empty((64, TPAD), np.float32)
    csa64[0::2] = cosT
    csa64[1::2] = cosT
    csb64[0::2] = -sinT
    csb64[1::2] = sinT
    CSA = np.concatenate([csa64, csa64], 0)
    CSB = np.concatenate([csb64, csb64], 0)

    ones4 = np.zeros((TPAD, NKV), ml_dtypes.bfloat16)
    ones4[:T] = 1.0

    jj = np.arange(128)[:, None]
    ii = np.arange(128)[None, :]
    mlo = (ii <= jj).astype(ml_dtypes.bfloat16)
    mup = (ii >= jj).astype(ml_dtypes.bfloat16)
    bd = np.zeros((128, 2), np.float32)
    bd[:64, 0] = 0.125
    bd[64:, 1] = 0.125
    bc2 = np.zeros((2, 128), np.float32)
    bc2[0, :64] = 1.0
    bc2[1, 64:] = 1.0

    in_maps = []
    for c in range(8):
        b, ci = c // 4, c % 4
        q0 = CH * ci
        in_maps.append({
            "xt": np.ascontiguousarray(XT[b][:, q0:q0 + NK]),
            "wqt": wq_t, "wkt": wk_t, "wvt": wv_t, "wgt": wg_t, "wpt": wp_t,
            "csa": np.ascontiguousarray(CSA[:, q0:q0 + NK]),
            "csb": np.ascontiguousarray(CSB[:, q0:q0 + NK]),
            "ve": np.ascontiguousarray(VEP[b][q0:q0 + NK]),
            "ones4": np.ascontiguousarray(ones4[q0:q0 + NK]),
            "mlo": mlo, "mup": mup, "bd": bd, "bc2": bc2,
        })
    return in_maps


def kernel(**inputs):
    global _COMPILED
    if _COMPILED is None:
        _COMPILED = build_program()
    nc = _COMPILED
    in_maps = host_prep(inputs)

    from concourse.bass_utils import run_bass_kernel_spmd
    res = run_bass_kernel_spmd(nc, in_maps, list(range(8)))

    out = np.empty((B, T, NE), np.float32)
    for c in range(8):
        b, ci = c // 4, c % 4
        out[b, CH * ci:CH * ci + CH] = res.results[c]["out"]
    return out

